# revision 17
# baseline (speedup 1.0000x reference)
"""Trainium2 Bass kernel for nn_Attention_90125593739547.

Full-input contract: kernel(**inputs) takes the unsharded numpy inputs and
returns the full [S, B, D] output. Internally:
  - 8 NeuronCores, core c handles batch b = c // 4 and 4 heads (c % 4).
  - Per-core program (all matmuls fp32r, transposed-scores attention):
      xT = x.T (PE transpose)                       [1024d, 2048t]
      qT = Wq.T @ xT + bq; kT = Wk.T @ xT + bk      [256hd, 2048]
      V~ = x @ [Wv | 0] + [bv | 1]                  [2048t, 4*65]  (ones col)
      per head pair, per q-half:
        St = kT_h.T-slice @ qT_h-slice (row-packed pairs)  [128t, 2*1024]
        pT = exp(SCALE * St)            (ScalarE, fp32r out)
        pv[65, 1024] += V~_h.T @ pT_h   (row 64 accumulates softmax denom)
        OT = pv[0:64] * recip(bcast(denom))   (K=1 ones matmul broadcast)
      y_partial = OT.T @ Wp_slice + bp_share        [2048, 1024]
  - Host sums the 4 per-head-group partials per batch (tensor-parallel reduce).
"""
import sys
sys.path.insert(0, '/opt/trn_rl_repo')
import numpy as np
from contextlib import ExitStack

S, B, D = 2048, 2, 1024
H, HD = 16, 64
SCALE = 1.0 / (HD ** 0.5)
P = 128
N_CORES = 8
CORES_PER_B = 4
NH = H // CORES_PER_B          # heads per core = 4
HDL = NH * HD                  # local head width = 256
SQ = S                         # q span per core (full sequence)
QH = 1024                      # q processed per attention stripe

_cache = {}


def _build():
    import concourse.bacc as bacc
    import concourse.mybir as mybir
    from concourse import tile

    F32 = mybir.dt.float32
    F32R = mybir.dt.float32r
    AF = mybir.ActivationFunctionType

    n_d, n_t, n_m = D // P, S // P, HDL // P
    n_qh = SQ // QH
    NV = NH * 65

    nc = bacc.Bacc("TRN2", target_bir_lowering=False, debug=False,
                   num_devices=N_CORES)

    x = nc.dram_tensor("x", [S, D], F32R, kind="ExternalInput")
    wq = nc.dram_tensor("wq", [D, HDL], F32R, kind="ExternalInput")
    wk = nc.dram_tensor("wk", [D, HDL], F32R, kind="ExternalInput")
    wv = nc.dram_tensor("wv", [D, NV], F32R, kind="ExternalInput")
    bq = nc.dram_tensor("bq", [1, HDL], F32R, kind="ExternalInput")
    bk = nc.dram_tensor("bk", [1, HDL], F32R, kind="ExternalInput")
    bv = nc.dram_tensor("bv", [1, NV], F32R, kind="ExternalInput")
    wp = nc.dram_tensor("wp", [HDL, D], F32R, kind="ExternalInput")
    bp = nc.dram_tensor("bp", [1, D], F32R, kind="ExternalInput")
    ident_d = nc.dram_tensor("ident", [P, P], F32R, kind="ExternalInput")
    ones_d = nc.dram_tensor("ones", [1, 512], F32R, kind="ExternalInput")
    y = nc.dram_tensor("y", [SQ, D], F32, kind="ExternalOutput")

    with tile.TileContext(nc) as tc, ExitStack() as ctx:
        const = ctx.enter_context(tc.tile_pool(name="const", bufs=1))
        ident = const.tile([P, P], F32R)
        ones_row_t = const.tile([1, 512], F32R)
        nc.sync.dma_start(ident[:], ident_d[:, :])
        nc.sync.dma_start(ones_row_t[:], ones_d[:, :])
        ones_row = ones_row_t[:]

        kv_pool = ctx.enter_context(tc.tile_pool(name="kv", bufs=1))
        qT = [kv_pool.tile([P, SQ], F32R, tag=f"qT{m}", name=f"qT{m}") for m in range(n_m)]
        kT = [kv_pool.tile([P, S], F32R, tag=f"kT{m}", name=f"kT{m}") for m in range(n_m)]
        Vt = [kv_pool.tile([P, NV], F32R, tag=f"V{t}", name=f"V{t}") for t in range(n_t)]
        OT = [kv_pool.tile([P, SQ], F32R, tag=f"OT{m}", name=f"OT{m}") for m in range(n_m)]

        proj_pool = ctx.enter_context(tc.tile_pool(name="proj", bufs=1))
        wp_sb = [proj_pool.tile([P, D], F32R, tag=f"wp{m}", name=f"wp{m}")
                 for m in range(n_m)]
        for m in range(n_m):
            nc.sync.dma_start(wp_sb[m][:], wp[m * P:(m + 1) * P, :])
        ystream = ctx.enter_context(tc.tile_pool(name="ystream", bufs=3))

        bias_pool = ctx.enter_context(tc.tile_pool(name="bias", bufs=1))
        bq_t = bias_pool.tile([1, HDL], F32R, tag="bq", name="bq")
        bk_t = bias_pool.tile([1, HDL], F32R, tag="bk", name="bk")
        bv_t = bias_pool.tile([1, NV], F32R, tag="bv", name="bv")
        bp_t = bias_pool.tile([1, D], F32R, tag="bp", name="bp")
        nc.sync.dma_start(bq_t[:], bq[:, :])
        nc.sync.dma_start(bk_t[:], bk[:, :])
        nc.sync.dma_start(bv_t[:], bv[:, :])
        nc.sync.dma_start(bp_t[:], bp[:, :])

        # ---- Phases A+B: transpose + QKV (xT/weights freed afterwards) ----
        with tc.tile_pool(name="xw", bufs=1) as xw_pool, \
             tc.tile_pool(name="stream", bufs=3) as stream, \
             tc.tile_pool(name="psumAB", bufs=1, space="PSUM") as psum:
            xT = [xw_pool.tile([P, S], F32R, tag=f"xT{d}", name=f"xT{d}") for d in range(n_d)]
            wq_sb = [xw_pool.tile([P, HDL], F32R, tag=f"wq{d}", name=f"wq{d}") for d in range(n_d)]
            wk_sb = [xw_pool.tile([P, HDL], F32R, tag=f"wk{d}", name=f"wk{d}") for d in range(n_d)]
            wv_sb = [xw_pool.tile([P, NV], F32R, tag=f"wv{d}", name=f"wv{d}") for d in range(n_d)]
            for d in range(n_d):
                nc.sync.dma_start(wq_sb[d][:], wq[d * P:(d + 1) * P, :])
                nc.sync.dma_start(wk_sb[d][:], wk[d * P:(d + 1) * P, :])
                nc.sync.dma_start(wv_sb[d][:], wv[d * P:(d + 1) * P, :])

            # A: x transpose (evictions alternate DVE/ACT)
            for st in range(n_t):
                xt_in = stream.tile([P, D], F32R, tag="x_in", name="x_in")
                nc.sync.dma_start(xt_in[:], x[st * P:(st + 1) * P, :])
                for dt in range(n_d):
                    ps = psum.tile([P, P], F32R, tag="tr", name="tr", bufs=2)
                    nc.tensor.transpose(ps[:], xt_in[:, dt * P:(dt + 1) * P], ident[:])
                    if dt % 2 == 0:
                        nc.vector.tensor_copy(xT[dt][:, st * P:(st + 1) * P], ps[:])
                    else:
                        nc.scalar.copy(xT[dt][:, st * P:(st + 1) * P], ps[:])

            # B: V~ = x @ [Wv|0] + [bv|1]  (first: attention needs all of V)
            for tt in range(n_t):
                ps = psum.tile([P, NV], F32, tag="qkv2", name="qkv2", bufs=2)
                for lo in range(0, NV, 512):
                    w = min(512, NV - lo)
                    for dt in range(n_d):
                        nc.tensor.matmul(ps[:, lo:lo + w],
                                         xT[dt][:, tt * P:(tt + 1) * P],
                                         wv_sb[dt][:, lo:lo + w],
                                         start=(dt == 0), stop=False)
                    nc.tensor.matmul(ps[:, lo:lo + w], ones_row[0:1, 0:P],
                                     bv_t[0:1, lo:lo + w], start=False, stop=True)
                nc.vector.tensor_copy(Vt[tt][:], ps[:])

            # B: qT, kT (+bias via K=1 ones matmul), 512-wide chains
            kqi = 0
            for m in range(n_m):
                for dst, wsb, bias in ((kT, wk_sb, bk_t), (qT, wq_sb, bq_t)):
                    for lo in range(0, S, 512):
                        ps = psum.tile([P, 512], F32, tag="qkv", name="qkv", bufs=4)
                        for dt in range(n_d):
                            nc.tensor.matmul(
                                ps[:], wsb[dt][:, m * P:(m + 1) * P],
                                xT[dt][:, lo:lo + 512],
                                start=(dt == 0), stop=False)
                        nc.tensor.matmul(ps[:], bias[0:1, m * P:(m + 1) * P],
                                         ones_row[0:1, 0:512],
                                         start=False, stop=True)
                        if kqi % 2 == 0:
                            nc.vector.tensor_copy(dst[m][:, lo:lo + 512], ps[:])
                        else:
                            nc.scalar.copy(dst[m][:, lo:lo + 512], ps[:])
                        kqi += 1

        # ---- Phase C: attention ----
        # stripes: (q-block of 512) major, head-pair minor -> projection can
        # start on finished q-blocks while later stripes still run.
        QB = 512
        with tc.tile_pool(name="attn", bufs=2) as attn_pool, \
             tc.tile_pool(name="psumC", bufs=1, space="PSUM") as psum:
            for qb in range(SQ // QB):
                qlo = qb * QB
                for m in range(n_m):
                    pvA = psum.tile([65, QB], F32, tag="pvA", name="pvA", bufs=2)
                    pvB = psum.tile([65, QB], F32, tag="pvB", name="pvB", bufs=1)
                    for tt in range(n_t):
                        sc = psum.tile([P, 2 * QB], F32, tag="sc", name="sc", bufs=2)
                        for half, plo in ((0, 0), (1, 64)):
                            nc.tensor.matmul(
                                sc[:, half * QB: half * QB + QB],
                                kT[m][plo:plo + 64, tt * P:(tt + 1) * P],
                                qT[m][plo:plo + 64, qlo: qlo + QB],
                                start=True, stop=True)
                        pT = attn_pool.tile([P, 2 * QB], F32R, tag="pT", name="pT",
                                            bufs=3)
                        nc.scalar.activation(pT[:], sc[:], AF.Exp, scale=SCALE)
                        for half, pv in ((0, pvA), (1, pvB)):
                            h = 2 * m + half
                            nc.tensor.matmul(
                                pv[:], Vt[tt][:, h * 65:(h + 1) * 65],
                                pT[:, half * QB: half * QB + QB],
                                start=(tt == 0), stop=(tt == n_t - 1))
                    for half, pv in ((0, pvA), (1, pvB)):
                        plo = half * 64
                        den = attn_pool.tile([1, QB], F32, tag="den", name="den")
                        nc.vector.tensor_copy(den[:], pv[64:65, :])
                        dnb = attn_pool.tile([64, QB], F32, tag="dnb", name="dnb")
                        nc.gpsimd.partition_broadcast(dnb[:], den[0:1, :])
                        rcb = attn_pool.tile([64, QB], F32, tag="rcb", name="rcb")
                        nc.vector.reciprocal_approx_fast(rcb[:], dnb[:])
                        nc.vector.tensor_tensor(
                            OT[m][plo:plo + 64, qlo:qlo + QB],
                            pv[0:64, :], rcb[:], op=mybir.AluOpType.mult)

            # ---- Phase D: projection (inside C psum pool; shares pvA slots) ----
            for qt in range(SQ // P):
                for nn in range(0, D, 512):
                    ps = psum.tile([P, 512], F32, tag="y", name="y", bufs=1)
                    for m in range(n_m):
                        nc.tensor.matmul(ps[:], OT[m][:, qt * P:(qt + 1) * P],
                                         wp_sb[m][:, nn:nn + 512],
                                         start=(m == 0), stop=False)
                    nc.tensor.matmul(ps[:], ones_row[0:1, 0:P], bp_t[0:1, nn:nn + 512],
                                     start=False, stop=True)
                    yt = ystream.tile([P, 512], F32, tag="y_out", name="y_out")
                    nc.vector.tensor_copy(yt[:], ps[:])
                    nc.sync.dma_start(y[qt * P:(qt + 1) * P, nn:nn + 512], yt[:])

    nc.compile()
    return nc


def _get_nc():
    if "nc" not in _cache:
        _cache["nc"] = _build()
    return _cache["nc"]


def make_in_maps(inputs, Wkv, bkv, Wq, bq, Wp, bp):
    """Host-side sharding: per-core input dicts."""
    inputs = np.asarray(inputs, dtype=np.float32)
    Wkv = np.asarray(Wkv, dtype=np.float32)
    bkv = np.asarray(bkv, dtype=np.float32)
    Wq = np.asarray(Wq, dtype=np.float32)
    bq = np.asarray(bq, dtype=np.float32)
    Wp = np.asarray(Wp, dtype=np.float32)
    bp = np.asarray(bp, dtype=np.float32)

    ident_np = np.eye(P, dtype=np.float32)
    ones_np = np.ones((1, 512), dtype=np.float32)
    bp_np = bp.reshape(1, D)
    zeros_bp = np.zeros((1, D), dtype=np.float32)

    in_maps = []
    for c in range(N_CORES):
        b = c // CORES_PER_B
        g = c % CORES_PER_B
        hsl = slice(g * HDL, (g + 1) * HDL)
        x_b = np.ascontiguousarray(inputs[:, b, :])
        wq_c = np.ascontiguousarray(Wq[:, hsl])
        bq_c = np.ascontiguousarray(bq[hsl]).reshape(1, HDL)
        wk_c = np.ascontiguousarray(Wkv[:, hsl])
        bk_c = np.ascontiguousarray(bkv[hsl]).reshape(1, HDL)
        wv_full = Wkv[:, H * HD + g * HDL: H * HD + (g + 1) * HDL]
        bv_full = bkv[H * HD + g * HDL: H * HD + (g + 1) * HDL]
        wv_c = np.zeros((D, NH * 65), dtype=np.float32)
        bv_c = np.zeros((1, NH * 65), dtype=np.float32)
        for h in range(NH):
            wv_c[:, h * 65:h * 65 + 64] = wv_full[:, h * 64:(h + 1) * 64]
            bv_c[0, h * 65:h * 65 + 64] = bv_full[h * 64:(h + 1) * 64]
            bv_c[0, h * 65 + 64] = 1.0
        wp_c = np.ascontiguousarray(Wp[hsl, :])
        in_maps.append(dict(
            x=x_b, wq=wq_c, wk=wk_c, wv=wv_c, bq=bq_c, bk=bk_c, bv=bv_c,
            wp=wp_c, bp=(bp_np if g == 0 else zeros_bp),
            ident=ident_np, ones=ones_np))
    return in_maps


def combine_outputs(results):
    """Host-side unshard: sum head-group partials per batch."""
    out = np.zeros((S, B, D), dtype=np.float32)
    for b in range(B):
        acc = results[b * CORES_PER_B]["y"].copy()
        for g in range(1, CORES_PER_B):
            acc += results[b * CORES_PER_B + g]["y"]
        out[:, b, :] = acc
    return out


def kernel(inputs, Wkv, bkv, Wq, bq, Wp, bp):
    from concourse.bass_utils import run_bass_kernel_spmd
    nc = _get_nc()
    in_maps = make_in_maps(inputs, Wkv, bkv, Wq, bq, Wp, bp)
    res = run_bass_kernel_spmd(nc, in_maps, list(range(N_CORES)))
    return combine_outputs(res.results)


# revision 20
# speedup vs baseline: 1.2374x; 1.2374x over previous
"""Trainium2 Bass kernel for nn_Attention_90125593739547.

Full-input contract: kernel(**inputs) takes the unsharded numpy inputs and
returns the full [S, B, D] output. Internally:
  - 8 NeuronCores, core c handles batch b = c // 4 and 4 heads (c % 4).
  - Per-core program (all matmuls fp32r, transposed-scores attention):
      xT = x.T (PE transpose)                       [1024d, 2048t]
      qT = Wq.T @ xT + bq; kT = Wk.T @ xT + bk      [256hd, 2048]
      V~ = x @ [Wv | 0] + [bv | 1]                  [2048t, 4*65]  (ones col)
      per head pair, per q-half:
        St = kT_h.T-slice @ qT_h-slice (row-packed pairs)  [128t, 2*1024]
        pT = exp(SCALE * St)            (ScalarE, fp32r out)
        pv[65, 1024] += V~_h.T @ pT_h   (row 64 accumulates softmax denom)
        OT = pv[0:64] * recip(bcast(denom))   (K=1 ones matmul broadcast)
      y_partial = OT.T @ Wp_slice + bp_share        [2048, 1024]
  - Host sums the 4 per-head-group partials per batch (tensor-parallel reduce).
"""
import sys
sys.path.insert(0, '/opt/trn_rl_repo')
import numpy as np
from contextlib import ExitStack

S, B, D = 2048, 2, 1024
H, HD = 16, 64
SCALE = 1.0 / (HD ** 0.5)
P = 128
N_CORES = 8
CORES_PER_B = 4
NH = H // CORES_PER_B          # heads per core = 4
HDL = NH * HD                  # local head width = 256
SQ = S                         # q span per core (full sequence)
QH = 1024                      # q processed per attention stripe

_cache = {}


def _build():
    import concourse.bacc as bacc
    import concourse.mybir as mybir
    from concourse import tile

    F32 = mybir.dt.float32
    F32R = mybir.dt.float32r
    AF = mybir.ActivationFunctionType

    n_d, n_t, n_m = D // P, S // P, HDL // P
    n_qh = SQ // QH
    NV = NH * 65

    nc = bacc.Bacc("TRN2", target_bir_lowering=False, debug=False,
                   num_devices=N_CORES)

    x = nc.dram_tensor("x", [S, D], F32R, kind="ExternalInput")
    wq = nc.dram_tensor("wq", [D, HDL], F32R, kind="ExternalInput")
    wk = nc.dram_tensor("wk", [D, HDL], F32R, kind="ExternalInput")
    wv = nc.dram_tensor("wv", [D, NV], F32R, kind="ExternalInput")
    bq = nc.dram_tensor("bq", [1, HDL], F32R, kind="ExternalInput")
    bk = nc.dram_tensor("bk", [1, HDL], F32R, kind="ExternalInput")
    bv = nc.dram_tensor("bv", [1, NV], F32R, kind="ExternalInput")
    wp = nc.dram_tensor("wp", [HDL, D], F32R, kind="ExternalInput")
    bp = nc.dram_tensor("bp", [1, D], F32R, kind="ExternalInput")
    ident_d = nc.dram_tensor("ident", [P, P], F32R, kind="ExternalInput")
    ones_d = nc.dram_tensor("ones", [1, 512], F32R, kind="ExternalInput")
    y = nc.dram_tensor("y", [SQ, D], F32, kind="ExternalOutput")

    with tile.TileContext(nc) as tc, ExitStack() as ctx:
        const = ctx.enter_context(tc.tile_pool(name="const", bufs=1))
        ident = const.tile([P, P], F32R)
        ones_row_t = const.tile([1, 512], F32R)
        nc.sync.dma_start(ident[:], ident_d[:, :])
        nc.sync.dma_start(ones_row_t[:], ones_d[:, :])
        ones_row = ones_row_t[:]

        kv_pool = ctx.enter_context(tc.tile_pool(name="kv", bufs=1))
        qT = [kv_pool.tile([P, SQ], F32R, tag=f"qT{m}", name=f"qT{m}") for m in range(n_m)]
        kT = [kv_pool.tile([P, S], F32R, tag=f"kT{m}", name=f"kT{m}") for m in range(n_m)]
        Vt = [kv_pool.tile([P, NV], F32R, tag=f"V{t}", name=f"V{t}") for t in range(n_t)]
        OT = [kv_pool.tile([P, SQ], F32R, tag=f"OT{m}", name=f"OT{m}") for m in range(n_m)]

        proj_pool = ctx.enter_context(tc.tile_pool(name="proj", bufs=1))
        wp_sb = [proj_pool.tile([P, D], F32R, tag=f"wp{m}", name=f"wp{m}")
                 for m in range(n_m)]
        for m in range(n_m):
            nc.sync.dma_start(wp_sb[m][:], wp[m * P:(m + 1) * P, :])
        ystream = ctx.enter_context(tc.tile_pool(name="ystream", bufs=3))

        bias_pool = ctx.enter_context(tc.tile_pool(name="bias", bufs=1))
        bq_t = bias_pool.tile([1, HDL], F32R, tag="bq", name="bq")
        bk_t = bias_pool.tile([1, HDL], F32R, tag="bk", name="bk")
        bv_t = bias_pool.tile([1, NV], F32R, tag="bv", name="bv")
        bp_t = bias_pool.tile([1, D], F32R, tag="bp", name="bp")
        nc.sync.dma_start(bq_t[:], bq[:, :])
        nc.sync.dma_start(bk_t[:], bk[:, :])
        nc.sync.dma_start(bv_t[:], bv[:, :])
        nc.sync.dma_start(bp_t[:], bp[:, :])

        # ---- Phases A+B: transpose + QKV (xT/weights freed afterwards) ----
        with tc.tile_pool(name="xw", bufs=1) as xw_pool, \
             tc.tile_pool(name="stream", bufs=3) as stream, \
             tc.tile_pool(name="psumAB", bufs=1, space="PSUM") as psum:
            xT = [xw_pool.tile([P, S], F32R, tag=f"xT{d}", name=f"xT{d}") for d in range(n_d)]
            wq_sb = [xw_pool.tile([P, HDL], F32R, tag=f"wq{d}", name=f"wq{d}") for d in range(n_d)]
            wk_sb = [xw_pool.tile([P, HDL], F32R, tag=f"wk{d}", name=f"wk{d}") for d in range(n_d)]
            wv_sb = [xw_pool.tile([P, NV], F32R, tag=f"wv{d}", name=f"wv{d}") for d in range(n_d)]
            # A: x transpose (evictions alternate DVE/ACT); x DMAs issued first
            for st in range(n_t):
                xt_in = stream.tile([P, D], F32R, tag="x_in", name="x_in")
                nc.sync.dma_start(xt_in[:], x[st * P:(st + 1) * P, :])
                if st == 3:
                    # weights follow the first few x tiles in the DMA queues
                    for d in range(n_d):
                        nc.sync.dma_start(wv_sb[d][:], wv[d * P:(d + 1) * P, :])
                        nc.sync.dma_start(wk_sb[d][:], wk[d * P:(d + 1) * P, :])
                        nc.sync.dma_start(wq_sb[d][:], wq[d * P:(d + 1) * P, :])
                for dt in range(n_d):
                    ps = psum.tile([P, P], F32R, tag="tr", name="tr", bufs=2)
                    nc.tensor.transpose(ps[:], xt_in[:, dt * P:(dt + 1) * P], ident[:])
                    if dt % 2 == 0:
                        nc.vector.tensor_copy(xT[dt][:, st * P:(st + 1) * P], ps[:])
                    else:
                        nc.scalar.copy(xT[dt][:, st * P:(st + 1) * P], ps[:])

            # B: V~ = x @ [Wv|0] + [bv|1]  (first: attention needs all of V)
            for tt in range(n_t):
                ps = psum.tile([P, NV], F32, tag="qkv2", name="qkv2", bufs=2)
                for lo in range(0, NV, 512):
                    w = min(512, NV - lo)
                    for dt in range(n_d):
                        nc.tensor.matmul(ps[:, lo:lo + w],
                                         xT[dt][:, tt * P:(tt + 1) * P],
                                         wv_sb[dt][:, lo:lo + w],
                                         start=(dt == 0), stop=False)
                    nc.tensor.matmul(ps[:, lo:lo + w], ones_row[0:1, 0:P],
                                     bv_t[0:1, lo:lo + w], start=False, stop=True)
                nc.vector.tensor_copy(Vt[tt][:], ps[:])

            # B: qT, kT (+bias via K=1 ones matmul), 512-wide chains
            kqi = 0
            for m in range(n_m):
                for dst, wsb, bias in ((kT, wk_sb, bk_t), (qT, wq_sb, bq_t)):
                    for lo in range(0, S, 512):
                        ps = psum.tile([P, 512], F32, tag="qkv", name="qkv", bufs=4)
                        for dt in range(n_d):
                            nc.tensor.matmul(
                                ps[:], wsb[dt][:, m * P:(m + 1) * P],
                                xT[dt][:, lo:lo + 512],
                                start=(dt == 0), stop=False)
                        nc.tensor.matmul(ps[:], bias[0:1, m * P:(m + 1) * P],
                                         ones_row[0:1, 0:512],
                                         start=False, stop=True)
                        if kqi % 2 == 0:
                            nc.vector.tensor_copy(dst[m][:, lo:lo + 512], ps[:])
                        else:
                            nc.scalar.copy(dst[m][:, lo:lo + 512], ps[:])
                        kqi += 1

        # ---- Phase C: attention ----
        # stripes: (q-block of 512) major, head-pair minor -> projection can
        # start on finished q-blocks while later stripes still run.
        QB = 512
        with tc.tile_pool(name="attn", bufs=2) as attn_pool, \
             tc.tile_pool(name="psumC", bufs=1, space="PSUM") as psum:
            for qb in range(SQ // QB):
                qlo = qb * QB
                for m in range(n_m):
                    pvA = psum.tile([65, QB], F32, tag="pvA", name="pvA", bufs=2)
                    pvB = psum.tile([65, QB], F32, tag="pvB", name="pvB", bufs=1)
                    for tt in range(n_t):
                        sc = psum.tile([P, 2 * QB], F32, tag="sc", name="sc", bufs=2)
                        for half, plo in ((0, 0), (1, 64)):
                            nc.tensor.matmul(
                                sc[:, half * QB: half * QB + QB],
                                kT[m][plo:plo + 64, tt * P:(tt + 1) * P],
                                qT[m][plo:plo + 64, qlo: qlo + QB],
                                start=True, stop=True)
                        pT = attn_pool.tile([P, 2 * QB], F32R, tag="pT", name="pT",
                                            bufs=3)
                        nc.scalar.activation(pT[:], sc[:], AF.Exp, scale=SCALE)
                        for half, pv in ((0, pvA), (1, pvB)):
                            h = 2 * m + half
                            nc.tensor.matmul(
                                pv[:], Vt[tt][:, h * 65:(h + 1) * 65],
                                pT[:, half * QB: half * QB + QB],
                                start=(tt == 0), stop=(tt == n_t - 1))
                    for half, pv in ((0, pvA), (1, pvB)):
                        plo = half * 64
                        den = attn_pool.tile([1, QB], F32, tag="den", name="den")
                        nc.vector.tensor_copy(den[:], pv[64:65, :])
                        dnb = attn_pool.tile([64, QB], F32, tag="dnb", name="dnb")
                        nc.gpsimd.partition_broadcast(dnb[:], den[0:1, :])
                        rcb = attn_pool.tile([64, QB], F32, tag="rcb", name="rcb")
                        nc.vector.reciprocal_approx_fast(rcb[:], dnb[:])
                        nc.vector.tensor_tensor(
                            OT[m][plo:plo + 64, qlo:qlo + QB],
                            pv[0:64, :], rcb[:], op=mybir.AluOpType.mult)

            # ---- Phase D: projection ----
            for qt in range(SQ // P):
                for nn in range(0, D, 512):
                    ps = psum.tile([P, 512], F32, tag="y", name="y", bufs=1)
                    for m in range(n_m):
                        nc.tensor.matmul(ps[:], OT[m][:, qt * P:(qt + 1) * P],
                                         wp_sb[m][:, nn:nn + 512],
                                         start=(m == 0), stop=False)
                    nc.tensor.matmul(ps[:], ones_row[0:1, 0:P], bp_t[0:1, nn:nn + 512],
                                     start=False, stop=True)
                    yt = ystream.tile([P, 512], F32, tag="y_out", name="y_out")
                    nc.vector.tensor_copy(yt[:], ps[:])
                    nc.sync.dma_start(y[qt * P:(qt + 1) * P, nn:nn + 512], yt[:])

    nc.compile()
    return nc


def _get_nc():
    if "nc" not in _cache:
        _cache["nc"] = _build()
    return _cache["nc"]


def make_in_maps(inputs, Wkv, bkv, Wq, bq, Wp, bp):
    """Host-side sharding: per-core input dicts."""
    inputs = np.asarray(inputs, dtype=np.float32)
    Wkv = np.asarray(Wkv, dtype=np.float32)
    bkv = np.asarray(bkv, dtype=np.float32)
    Wq = np.asarray(Wq, dtype=np.float32)
    bq = np.asarray(bq, dtype=np.float32)
    Wp = np.asarray(Wp, dtype=np.float32)
    bp = np.asarray(bp, dtype=np.float32)

    ident_np = np.eye(P, dtype=np.float32)
    ones_np = np.ones((1, 512), dtype=np.float32)
    bp_np = bp.reshape(1, D)
    zeros_bp = np.zeros((1, D), dtype=np.float32)

    in_maps = []
    for c in range(N_CORES):
        b = c // CORES_PER_B
        g = c % CORES_PER_B
        hsl = slice(g * HDL, (g + 1) * HDL)
        x_b = np.ascontiguousarray(inputs[:, b, :])
        wq_c = np.ascontiguousarray(Wq[:, hsl])
        bq_c = np.ascontiguousarray(bq[hsl]).reshape(1, HDL)
        wk_c = np.ascontiguousarray(Wkv[:, hsl])
        bk_c = np.ascontiguousarray(bkv[hsl]).reshape(1, HDL)
        wv_full = Wkv[:, H * HD + g * HDL: H * HD + (g + 1) * HDL]
        bv_full = bkv[H * HD + g * HDL: H * HD + (g + 1) * HDL]
        wv_c = np.zeros((D, NH * 65), dtype=np.float32)
        bv_c = np.zeros((1, NH * 65), dtype=np.float32)
        for h in range(NH):
            wv_c[:, h * 65:h * 65 + 64] = wv_full[:, h * 64:(h + 1) * 64]
            bv_c[0, h * 65:h * 65 + 64] = bv_full[h * 64:(h + 1) * 64]
            bv_c[0, h * 65 + 64] = 1.0
        wp_c = np.ascontiguousarray(Wp[hsl, :])
        in_maps.append(dict(
            x=x_b, wq=wq_c, wk=wk_c, wv=wv_c, bq=bq_c, bk=bk_c, bv=bv_c,
            wp=wp_c, bp=(bp_np if g == 0 else zeros_bp),
            ident=ident_np, ones=ones_np))
    return in_maps


def combine_outputs(results):
    """Host-side unshard: sum head-group partials per batch."""
    out = np.zeros((S, B, D), dtype=np.float32)
    for b in range(B):
        acc = results[b * CORES_PER_B]["y"].copy()
        for g in range(1, CORES_PER_B):
            acc += results[b * CORES_PER_B + g]["y"]
        out[:, b, :] = acc
    return out


def kernel(inputs, Wkv, bkv, Wq, bq, Wp, bp):
    from concourse.bass_utils import run_bass_kernel_spmd
    nc = _get_nc()
    in_maps = make_in_maps(inputs, Wkv, bkv, Wq, bq, Wp, bp)
    res = run_bass_kernel_spmd(nc, in_maps, list(range(N_CORES)))
    return combine_outputs(res.results)
